# revision 25
# baseline (speedup 1.0000x reference)
"""Causal single-head attention (B=4, S=4096, d=1024) on 8 Trainium2 NeuronCores.

Sharding: 8 cores = 4 batches x 2 sequence-groups.  Per batch, the 8 causal
q-blocks of 512 rows (k-tile coverage 4,8,...,32) are paired so each core gets
72 real k-tile visits, padded to one uniform static program with slot
coverages (8,16,24,32) = 80 visits.  Causal masking and padding are handled by
a data-driven mask  A = exp(s/sqrt(d)) * (I - J <= delta)  so all 8 cores run
a single SPMD program; only input data differs per core.

v5: all matmul operands bf16 (2 cols/cycle on hw, ~126 ns per N=512 matmul);
kT [128,8,4096] and v [128,32,1024] are SBUF-resident for the whole kernel
(projection writes them directly via ACT PSUM->SBUF copies), so attention
issues no DMA except per-slot qT loads (slots 1+; slot 0's qT is written
straight to SBUF during projection) and output stores.  Normalization (1/den)
runs on DVE with stores on an idle HWDGE queue; ACT does only exp in
attention.  The attention inner loop is software-pipelined: visit i's
attn@v/denominator matmuls are emitted after visit i+1's score matmuls so the
PE never waits on the ACT exp.  PSUM fp32 throughout.

Math (per core):
  qT = Wq^T xq^T, kT = Wk^T x^T (both [d, s], d on partitions), v = x Wv.
  Per slot (512 q rows), per k-tile (128 rows):
    sT[k, q]   = sum_e kT[e,k] qT[e,q]          (PE, bf16, N=512)
    A[k, q]    = exp(sT/32) * mask              (ACT exp PSUM->SBUF bf16,
                                                 DVE mask via (ij<=dl)*A)
    out[q, d] += A[:,qm]^T v[k, d]              (PE, accumulated in PSUM)
    den[q]    += A[:,qm]^T ones                 (PE, N=2)
  out = out * (1/den)   (DVE reciprocal + DVE tensor_scalar_mul)
No running max is needed: scores are ~N(0,1) after the 1/32 scale, and exp
without max-subtraction is exact in fp32/bf16 range here.
"""

import contextlib
import math

import numpy as np

import concourse.bass as bass  # noqa: F401
import concourse.mybir as mybir
import concourse.tile as tile
from concourse import bacc
from concourse.bass_utils import run_bass_kernel_spmd

# Leave matmul sync-waits on the InstMatmult itself instead of migrating them
# onto the paired InstLdweights: waits attached to ldweights serialize the
# PE's weight-load pipeline (measured ~40% slower on matmul-dense streams).
# Multi-wait matmuls are still legalized by generate_event_semaphores.
bacc.Bacc.move_matmul_waits_to_ldweights = lambda self: None

F32 = mybir.dt.float32
BF16 = mybir.dt.bfloat16
NP_BF16 = mybir.dt.np(mybir.dt.bfloat16)
AF = mybir.ActivationFunctionType
ALU = mybir.AluOpType

CFG_FULL = dict(S=4096, D=1024, QBLK=512, COV=(8, 16, 24, 32))
Q0_FULL = {0: (0, 1536, 2048, 3584), 1: (512, 1024, 2560, 3072)}
RG_FULL = [[0, 1], [2, 3], [4, 5], [6, 7]]
B_FULL = 4
USE_RG = False

# Tuning knobs (A/B tested on hardware)
VARIANT = dict(
    store_q="act",      # queue for output stores: "gpsimd" | "act"
    w_q="gpsimd",       # queue for weight loads: "gpsimd" | "sync"
    copy_mix=False,     # alternate phase-1 PSUM->SBUF copies ACT/DVE
    staged_copy=False,  # route kT/v writes via small staging + SB->SB DMA
    den=True,           # compute denominator (False = timing probe only)
    wkchunk=False,      # chunk the Wk DMA for a faster start
    phase1_only=False,  # timing probe: skip attention entirely
    noexp=False,        # timing probe: consume a memset tile instead of exp
)


def build_nc(S, D, QBLK, COV, reps=1, rg=None, variant=None):
    """Build the single-core Bass program (identical across all cores)."""
    assert rg is None
    va = dict(VARIANT)
    if variant:
        va.update(variant)
    DC = D // 128
    M = QBLK // 128
    nslots = len(COV)
    QROWS = nslots * QBLK
    DHALF = min(512, D)
    NH = D // DHALF
    SBLK = min(512, S)
    NSB = S // SBLK
    maxcov = max(COV)
    assert maxcov == S // 128
    scale = 1.0 / math.sqrt(D)

    nc = bacc.Bacc("TRN2", target_bir_lowering=False)
    xT_d = nc.dram_tensor("xT", [D, S], BF16, kind="ExternalInput")
    xTq_d = nc.dram_tensor("xTq", [D, QROWS], BF16, kind="ExternalInput")
    wq_d = nc.dram_tensor("Wq", [D, D], BF16, kind="ExternalInput")
    wk_d = nc.dram_tensor("Wk", [D, D], BF16, kind="ExternalInput")
    wv_d = nc.dram_tensor("Wv", [D, D], BF16, kind="ExternalInput")
    ij_d = nc.dram_tensor("IJ", [128, QBLK], F32, kind="ExternalInput")
    dl_d = nc.dram_tensor("delta", [128, nslots * maxcov], F32,
                          kind="ExternalInput")
    ones_d = nc.dram_tensor("ones", [128, 2], BF16, kind="ExternalInput")
    out_d = nc.dram_tensor("out", [QROWS, D], F32, kind="ExternalOutput")

    def dpart(ap):
        return ap.rearrange("(c p) n -> p c n", p=128)

    with tile.TileContext(nc) as tc:
        with tc.tile_pool(name="dram", bufs=1, space="DRAM") as dram, \
             tc.tile_pool(name="persist", bufs=1) as persist:
            qT_i = dram.tile([DC, 128, QROWS], BF16, name="qT_i")
            kT_sb = persist.tile([128, DC, S], BF16, name="kT_sb", tag="kT")
            v_sb = persist.tile([128, S // 128, D], BF16, name="v_sb",
                                tag="v")
            qt0_sb = persist.tile([128, DC, QBLK], BF16, name="qt0_sb",
                                  tag="qt0")
            ij_sb = persist.tile([128, QBLK], F32, name="ij", tag="ij")
            dl_sb = persist.tile([128, nslots * maxcov], F32, name="dl",
                                 tag="dl")
            ones_sb = persist.tile([128, 2], BF16, name="ones", tag="ones")
            dummy_ps_box = [None]

            def touch(cols2):
                dummy_ps = dummy_ps_box[0]
                # Tiny matmul reading two columns of a freshly DMA'd SBUF
                # tile: self-loading matmuls allow one sync wait, so the PE
                # observes DMA ticks via these.
                nc.tensor.matmul(dummy_ps[0:1, 0:2], cols2[:, 0:1], cols2,
                                 start=True, stop=True)

            def store_dma(out_ap, in_ap):
                if va["store_q"] == "gpsimd":
                    nc.gpsimd.dma_start(out=out_ap, in_=in_ap)
                else:
                    nc.scalar.dma_start(out=out_ap, in_=in_ap)

            _loop = (tc.For_i(0, reps, 1) if reps > 1
                     else contextlib.nullcontext())
            with _loop:
                # ---------------- Phase 1: projections ----------------
                with (
                    tc.tile_pool(name="w", bufs=1) as wpool,
                    tc.tile_pool(name="xt", bufs=2) as xtpool,
                    tc.tile_pool(name="kst", bufs=5) as kspool,
                    tc.tile_pool(name="ppsum", bufs=7, space="PSUM") as ppsum,
                    tc.tile_pool(name="dummy", bufs=1,
                                 space="PSUM") as dummypool,
                ):
                    dummy_ps_box[0] = dummypool.tile(
                        [128, 2], F32, name="dummy_ps", tag="dummy")
                    wdma = (nc.gpsimd.dma_start if va["w_q"] == "gpsimd"
                            else nc.sync.dma_start)
                    w_sb = {}
                    for name in ("k", "v", "q"):
                        w_sb[name] = wpool.tile([128, DC, D], BF16,
                                                name=f"w{name}",
                                                tag=f"w{name}")
                    if va["wkchunk"]:
                        nc.sync.dma_start(out=w_sb["k"][:, 0, :],
                                          in_=dpart(wk_d[:, :])[:, 0, :])
                    else:
                        wdma(out=w_sb["k"], in_=dpart(wk_d[:, :]))
                    if va["w_q"] == "gpsimd":
                        wdma(out=w_sb["v"], in_=dpart(wv_d[:, :]))
                        wdma(out=w_sb["q"], in_=dpart(wq_d[:, :]))

                    kvjobs = [("kv", sb) for sb in range(NSB)]
                    qjobs = [("q", s) for s in range(nslots)]
                    jobs = []
                    per = max(1, NSB // nslots)
                    for s in range(nslots):
                        jobs += kvjobs[s * per:(s + 1) * per]
                        jobs.append(qjobs[s])
                    jobs += kvjobs[nslots * per:]

                    def xt_load(job):
                        kind, idx = job
                        blk = SBLK if kind == "kv" else QBLK
                        src = xT_d if kind == "kv" else xTq_d
                        xt = xtpool.tile([128, DC, blk], BF16, name="xt",
                                         tag="xt")
                        nc.sync.dma_start(
                            out=xt,
                            in_=dpart(src[:, idx * blk:(idx + 1) * blk]))
                        return xt

                    xts = {0: xt_load(jobs[0])}
                    if va["wkchunk"]:
                        for ci in range(1, DC):
                            nc.sync.dma_start(
                                out=w_sb["k"][:, ci, :],
                                in_=dpart(wk_d[:, :])[:, ci, :])
                    touched = set()

                    def touch_w(name):
                        if name not in touched:
                            touched.add(name)
                            touch(w_sb[name][:, 0, 0:2])

                    _ncopy = [0]

                    def pcopy(dst, src):
                        # alternate PSUM->SBUF copies between ACT and DVE so
                        # neither engine's instruction stream limits phase 1
                        _ncopy[0] += 1
                        if va["copy_mix"] and _ncopy[0] % 2 == 0:
                            nc.vector.tensor_copy(out=dst, in_=src)
                        else:
                            nc.scalar.copy(out=dst, in_=src)

                    def pcopy_big(dst, src):
                        # writes into the big resident tiles go via a small
                        # staging tile + SBUF->SBUF DMA: compute-engine
                        # stores into the large tiles measurably slow the
                        # concurrent PE matmul stream; DMA writes don't
                        if va["staged_copy"]:
                            stg = kspool.tile([128, 512], BF16, name="stg",
                                              tag="ks")
                            pcopy(stg, src)
                            nc.gpsimd.dma_start(out=dst, in_=stg)
                        else:
                            pcopy(dst, src)

                    for jidx, job in enumerate(jobs):
                        if jidx + 1 < len(jobs):
                            xts[jidx + 1] = xt_load(jobs[jidx + 1])
                        if jidx == 0 and va["w_q"] != "gpsimd":
                            nc.sync.dma_start(out=w_sb["v"],
                                              in_=dpart(wv_d[:, :]))
                            nc.sync.dma_start(out=w_sb["q"],
                                              in_=dpart(wq_d[:, :]))
                        if jidx == 1:
                            nc.sync.dma_start(out=ij_sb, in_=ij_d[:, :])
                            nc.sync.dma_start(out=dl_sb, in_=dl_d[:, :])
                            nc.sync.dma_start(out=ones_sb, in_=ones_d[:, :])
                        xt = xts.pop(jidx)
                        touch(xt[:, 0, 0:2])
                        kind, idx = job
                        if kind == "kv":
                            if not va["wkchunk"]:
                                touch_w("k")
                            for co in range(DC):
                                ps = ppsum.tile([128, SBLK], F32, name="pp",
                                                tag="pp")
                                for ci in range(DC):
                                    if va["wkchunk"] and jidx == 0 \
                                            and co == 0:
                                        touch(w_sb["k"][:, ci, 0:2])
                                    nc.tensor.matmul(
                                        ps,
                                        w_sb["k"][:, ci,
                                                  co * 128:(co + 1) * 128],
                                        xt[:, ci, :],
                                        start=(ci == 0), stop=(ci == DC - 1))
                                pcopy_big(kT_sb[:, co,
                                                idx * SBLK:(idx + 1) * SBLK],
                                          ps)
                            touch_w("v")
                            for m in range(SBLK // 128):
                                for h in range(NH):
                                    ps = ppsum.tile([128, DHALF], F32,
                                                    name="pp", tag="pp")
                                    for ci in range(DC):
                                        nc.tensor.matmul(
                                            ps,
                                            xt[:, ci, m * 128:(m + 1) * 128],
                                            w_sb["v"][:, ci, h * DHALF:
                                                      (h + 1) * DHALF],
                                            start=(ci == 0),
                                            stop=(ci == DC - 1))
                                    pcopy_big(
                                        v_sb[:, idx * (SBLK // 128) + m,
                                             h * DHALF:(h + 1) * DHALF], ps)
                        else:
                            touch_w("q")
                            for co in range(DC):
                                ps = ppsum.tile([128, QBLK], F32, name="pp",
                                                tag="pp")
                                for ci in range(DC):
                                    nc.tensor.matmul(
                                        ps,
                                        w_sb["q"][:, ci,
                                                  co * 128:(co + 1) * 128],
                                        xt[:, ci, :],
                                        start=(ci == 0), stop=(ci == DC - 1))
                                if idx == 0:
                                    # slot 0's qT goes straight to SBUF
                                    pcopy(qt0_sb[:, co, :], ps)
                                else:
                                    ks = kspool.tile([128, QBLK], BF16,
                                                     name="ks", tag="ks")
                                    pcopy(ks, ps)
                                    nc.scalar.dma_start(
                                        out=qT_i[co, :,
                                                 idx * QBLK:(idx + 1) * QBLK],
                                        in_=ks)

                # ---------------- Phase 2: attention ----------------
                with (
                    tc.tile_pool(name="qt", bufs=2) as qtpool,
                    tc.tile_pool(name="at", bufs=maxcov + 4) as apool,
                    tc.tile_pool(name="ot", bufs=6) as otpool,
                    tc.tile_pool(name="rc", bufs=2) as rcpool,
                    tc.tile_pool(name="spsum", bufs=3, space="PSUM") as spsum,
                    tc.tile_pool(name="opsum", bufs=M, space="PSUM") as opsum,
                    tc.tile_pool(name="dpsum", bufs=1, space="PSUM") as dpsum,
                ):
                    def qt_load(s):
                        qt = qtpool.tile([128, DC, QBLK], BF16, name="qt",
                                         tag="qt")
                        nc.sync.dma_start(
                            out=qt,
                            in_=qT_i[:, :, s * QBLK:(s + 1) * QBLK]
                            .rearrange("c p y -> p c y"))
                        return qt

                    qt_next = None
                    nslots_run = 0 if va["phase1_only"] else nslots
                    for s in range(nslots_run):
                        cov = COV[s]
                        if s == 0:
                            qt = qt0_sb
                        else:
                            qt = qt_next
                        if va["noexp"]:
                            at_const = apool.tile([128, QBLK], BF16,
                                                  name="at", tag="at")
                            nc.vector.memset(at_const, 0.001)
                        po = [opsum.tile([128, DHALF], F32, name="po",
                                         tag="po") for _ in range(M)]
                        pd = dpsum.tile([128, 2 * M], F32, name="pd",
                                        tag="pd")

                        def consume(i, at, dst):
                            # attn@v (d-half 0) + denominator for visit i
                            for m in range(M):
                                nc.tensor.matmul(
                                    dst[m],
                                    at[:, m * 128:(m + 1) * 128],
                                    v_sb[:, i, 0:DHALF],
                                    start=(i == 0), stop=(i == cov - 1))
                            if va["den"]:
                                for m in range(M):
                                    nc.tensor.matmul(
                                        pd[:, 2 * m:2 * m + 2],
                                        at[:, m * 128:(m + 1) * 128],
                                        ones_sb[:, :],
                                        start=(i == 0 and m == 0),
                                        stop=(i == cov - 1 and m == M - 1))

                        # ---- sweep 1: scores + exp + attn@v(d-half 0) ----
                        a_tiles = []
                        prev = None
                        for i in range(cov):
                            ps = spsum.tile([128, QBLK], F32, name="ps",
                                            tag="ps")
                            for ci in range(DC):
                                nc.tensor.matmul(
                                    ps,
                                    kT_sb[:, ci, i * 128:(i + 1) * 128],
                                    qt[:, ci, :],
                                    start=(ci == 0), stop=(ci == DC - 1))
                            if va["noexp"]:
                                at = at_const
                            else:
                                at = apool.tile([128, QBLK], BF16, name="at",
                                                tag="at")
                                nc.scalar.activation(
                                    out=at, in_=ps, func=AF.Exp, scale=scale)
                                if i >= cov - 2 * M:
                                    # at = (ij <= delta) * at, one DVE op
                                    nc.vector.scalar_tensor_tensor(
                                        out=at, in0=ij_sb,
                                        scalar=dl_sb[:, s * maxcov + i:
                                                     s * maxcov + i + 1],
                                        in1=at, op0=ALU.is_le, op1=ALU.mult)
                            a_tiles.append(at)
                            if prev is not None:
                                consume(prev, a_tiles[prev], po)
                            prev = i
                        consume(prev, a_tiles[prev], po)

                        rc = rcpool.tile([128, 2 * M], F32, name="rc",
                                         tag="rc")
                        if va["den"]:
                            nc.vector.reciprocal(out=rc, in_=pd)
                        else:
                            nc.vector.memset(rc, 1.0)

                        # prefetch next slot's qt while sweep 2 runs
                        if s + 1 < nslots:
                            qt_next = qt_load(s + 1)

                        def norm_store(pom, m, h):
                            # out = po * (1/den) on DVE
                            ot = otpool.tile([128, DHALF], F32, name="ot",
                                             tag="ot")
                            nc.vector.tensor_scalar_mul(
                                out=ot, in0=pom,
                                scalar1=rc[:, 2 * m:2 * m + 1])
                            store_dma(
                                out_d[s * QBLK + m * 128:
                                      s * QBLK + (m + 1) * 128,
                                      h * DHALF:(h + 1) * DHALF], ot)

                        for m in range(M):
                            norm_store(po[m], m, 0)

                        # ---- sweep 2: attn@v d-half 1, m-major so each
                        # block normalizes/stores while the next computes ----
                        for m in range(M):
                            po2m = opsum.tile([128, DHALF], F32, name="po",
                                              tag="po")
                            for i in range(cov):
                                nc.tensor.matmul(
                                    po2m,
                                    a_tiles[i][:, m * 128:(m + 1) * 128],
                                    v_sb[:, i, DHALF:2 * DHALF],
                                    start=(i == 0), stop=(i == cov - 1))
                            norm_store(po2m, m, 1)
    nc.compile()
    return nc


def host_core_inputs(x_b, Wq, Wk, Wv, q0s, S, D, QBLK, COV, half=None):
    """Input map for one core (bf16 operands)."""
    nslots = len(COV)
    maxcov = max(COV)
    xT = np.ascontiguousarray(x_b.T).astype(NP_BF16)
    xTq = np.ascontiguousarray(
        np.concatenate([x_b[q0:q0 + QBLK] for q0 in q0s], axis=0).T
    ).astype(NP_BF16)
    ij = (np.arange(128, dtype=np.float32)[:, None]
          - np.arange(QBLK, dtype=np.float32)[None, :])
    ij = np.ascontiguousarray(np.broadcast_to(ij, (128, QBLK)))
    delta = np.empty((nslots, maxcov), dtype=np.float32)
    for s, q0 in enumerate(q0s):
        delta[s, :] = q0 - 128.0 * np.arange(maxcov, dtype=np.float32)
    delta = np.ascontiguousarray(
        np.broadcast_to(delta.reshape(1, -1), (128, nslots * maxcov)))
    return {
        "xT": xT, "xTq": xTq,
        "Wq": np.ascontiguousarray(Wq, np.float32).astype(NP_BF16),
        "Wk": np.ascontiguousarray(Wk, np.float32).astype(NP_BF16),
        "Wv": np.ascontiguousarray(Wv, np.float32).astype(NP_BF16),
        "IJ": ij, "delta": delta,
        "ones": np.ones((128, 2), NP_BF16),
    }


_NC_CACHE = {}


def _get_nc(key, cfg, **kw):
    if key not in _NC_CACHE:
        _NC_CACHE[key] = build_nc(**cfg, **kw)
    return _NC_CACHE[key]


def run_full(x, Wq, Wk, Wv, trace=False, trace_cores=None):
    """Run the 8-core kernel on full inputs; returns (out, BassKernelResults)."""
    cfg = CFG_FULL
    S, D, QBLK, COV = cfg["S"], cfg["D"], cfg["QBLK"], cfg["COV"]
    x = np.asarray(x, np.float32)
    Wq = np.asarray(Wq, np.float32)
    Wk = np.asarray(Wk, np.float32)
    Wv = np.asarray(Wv, np.float32)
    B = x.shape[0]
    assert (B, x.shape[1], x.shape[2]) == (B_FULL, S, D)

    nc = _get_nc("full", cfg)
    in_maps = []
    for b in range(B):
        for h in range(2):
            in_maps.append(host_core_inputs(
                x[b], Wq, Wk, Wv, Q0_FULL[h], S, D, QBLK, COV))
    res = run_bass_kernel_spmd(
        nc, in_maps, list(range(2 * B)), trace=trace,
        trace_cores=trace_cores)
    out = np.empty((B, S, D), np.float32)
    for b in range(B):
        for h in range(2):
            o = np.asarray(res.results[2 * b + h]["out"])
            for s, q0 in enumerate(Q0_FULL[h]):
                out[b, q0:q0 + QBLK] = o[s * QBLK:(s + 1) * QBLK]
    return out, res


def kernel(x, Wq, Wk, Wv):
    out, _ = run_full(x, Wq, Wk, Wv)
    return out
